# revision 7
# baseline (speedup 1.0000x reference)
"""AgentCollisionLoss Trainium2 kernel.

Sharding: 8 cores = B(4) x i-half(2). Core c handles b = c//2 and the 32
i-agents [h*32, h*32+32) vs all 64 j-agents over all T=80 steps.

On-device per core:
  world disk coords wx/wy for all 64 agents  (partitions = j)
  loop over 16 i-pairs: partitions p = (i-parity)*64 + j, free = (t,k,l)
    dx = xi - xj, dy = yi - yj           (DVE tensor_tensor, fp32)
    sqx = dx^2, sqy = dy^2               (ACT Square)
    d2 = sqx + sqy                       (GPSIMD scalar_tensor_tensor)
    dmin2 = min over (k,l)               (DVE tensor_reduce)
  clamp, sqrt (ACT), q = dmin/pd, pen = min(q-1, 0), weight+sum
Host: slice inputs per core, sum the 8x[128] partials, negate.
"""

import numpy as np

import concourse.bass as bass
import concourse.bacc as bacc
import concourse.tile as tile
import concourse.mybir as mybir
from concourse import bass_utils

B, N, T, D = 4, 64, 80, 6
K = 5
NCORES = 8
BUFFER_DIST = 0.2
DECAY_RATE = 0.9
TK = T * K          # 400
TKL = T * K * K     # 2000
NPAIR = 16          # i-pairs per core (32 i-agents / 2)
FD = mybir.dt.float32

_CACHE = {}
_LAST_INMAPS = None


def _build():
    nc = bacc.Bacc("TRN2", target_bir_lowering=False, debug=False,
                   num_devices=NCORES)

    y_in = nc.dram_tensor("y_in", [N, T * D], FD, kind="ExternalInput").ap()
    len_in = nc.dram_tensor("len_in", [N, 1], FD, kind="ExternalInput").ap()
    wid_in = nc.dram_tensor("wid_in", [N, 1], FD, kind="ExternalInput").ap()
    frac_in = nc.dram_tensor("frac_in", [K], FD, kind="ExternalInput").ap()
    wt_in = nc.dram_tensor("wt_in", [128, NPAIR * T], FD,
                           kind="ExternalInput").ap()
    part_out = nc.dram_tensor("part_out", [128, 1], FD,
                              kind="ExternalOutput").ap()

    stage_x = nc.dram_tensor("stage_x", [N * TK], FD, kind="Internal").ap()
    stage_y = nc.dram_tensor("stage_y", [N * TK], FD, kind="Internal").ap()
    stage_r = nc.dram_tensor("stage_r", [N], FD, kind="Internal").ap()

    # the per-core i-half offset is baked per-core via h below; we build one
    # program (SPMD) so instead we bake h into the *host-provided* Wt and by
    # providing per-core inputs; the i-slice offset must be identical across
    # cores, so the host rotates the agent axis per core such that the
    # i-agents are always rows [0, 32).
    HOFF = 0

    with tile.TileContext(nc) as tc:
        with (
            tc.tile_pool(name="prep", bufs=1) as prep,
            tc.tile_pool(name="rep", bufs=1) as rep,
            tc.tile_pool(name="xiyi", bufs=6) as xiyi,
            tc.tile_pool(name="work", bufs=3) as work,
            tc.tile_pool(name="acc", bufs=1) as acc,
        ):
            # ---- stage 1: per-agent prep (partitions = j, 64) ----
            ytile = prep.tile([N, T * D], FD)
            nc.sync.dma_start(out=ytile, in_=y_in)
            def ycol(dcol):
                return bass.AP(tensor=ytile.tensor,
                               offset=ytile.offset + dcol,
                               ap=[ytile.ap[0], [D, T]])
            x_ap = ycol(0)
            ypos_ap = ycol(1)
            yaw_ap = ycol(4)

            ltile = prep.tile([N, 1], FD)
            wtile = prep.tile([N, 1], FD)
            nc.sync.dma_start(out=ltile, in_=len_in)
            nc.sync.dma_start(out=wtile, in_=wid_in)
            fr = prep.tile([N, K], FD)
            nc.sync.dma_start(
                out=fr,
                in_=bass.AP(tensor=frac_in.tensor, offset=0,
                            ap=[[0, N], [1, K]]))

            zero128 = prep.tile([128, 1], FD)
            nc.vector.memset(zero128, 0.0)
            pi2 = prep.tile([N, 1], FD)
            nc.vector.memset(pi2, float(np.pi / 2))
            cosT = prep.tile([N, T], FD)
            sinT = prep.tile([N, T], FD)
            nc.scalar.activation(out=cosT, in_=yaw_ap,
                                 func=mybir.ActivationFunctionType.Sin,
                                 bias=pi2, scale=1.0)
            nc.scalar.activation(out=sinT, in_=yaw_ap,
                                 func=mybir.ActivationFunctionType.Sin,
                                 bias=zero128[:N, :], scale=1.0)

            rad = prep.tile([N, 1], FD)
            nc.vector.tensor_scalar(out=rad, in0=wtile, scalar1=0.5,
                                    scalar2=0.0, op0=mybir.AluOpType.mult,
                                    op1=mybir.AluOpType.add)
            # cmax = l/2 - rad ; cmin = -cmax ; cent = cmin + (cmax-cmin)*frac
            cmax = prep.tile([N, 1], FD)
            nc.vector.scalar_tensor_tensor(out=cmax, in0=ltile, scalar=0.5,
                                           in1=rad,
                                           op0=mybir.AluOpType.mult,
                                           op1=mybir.AluOpType.subtract)
            # cent[j,l] = cmax * f2[l]   (host provides f2 = 2*frac-1)
            cent = prep.tile([N, K], FD)
            nc.vector.tensor_scalar(out=cent, in0=fr, scalar1=cmax,
                                    scalar2=0.0,
                                    op0=mybir.AluOpType.mult,
                                    op1=mybir.AluOpType.add)

            wx = prep.tile([N, TK], FD)
            wy = prep.tile([N, TK], FD)
            # wx = cent[j,l]*cos[j,t] + x[j,t]
            tmp = prep.tile([N, TK], FD)

            def bc_tl(src_t):   # [N,T] -> (t,l) view
                return bass.AP(tensor=src_t.tensor, offset=src_t.offset,
                               ap=[src_t.ap[0], [src_t.ap[-1][0], T], [0, K]])

            def bc_lt(src_l):   # [N,K] -> (t,l) view
                return bass.AP(tensor=src_l.tensor, offset=src_l.offset,
                               ap=[src_l.ap[0], [0, T], [src_l.ap[-1][0], K]])

            wx3 = wx[:, :].rearrange("p (t l) -> p t l", l=K)
            wy3 = wy[:, :].rearrange("p (t l) -> p t l", l=K)
            tmp3 = tmp[:, :].rearrange("p (t l) -> p t l", l=K)

            nc.vector.tensor_tensor(out=tmp3, in0=bc_tl(cosT), in1=bc_lt(cent),
                                    op=mybir.AluOpType.mult)
            nc.vector.tensor_tensor(out=wx3, in0=tmp3, in1=bc_tl(x_ap),
                                    op=mybir.AluOpType.add)
            nc.vector.tensor_tensor(out=tmp3, in0=bc_tl(sinT), in1=bc_lt(cent),
                                    op=mybir.AluOpType.mult)
            nc.vector.tensor_tensor(out=wy3, in0=bc_tl(ypos_ap), in1=tmp3,
                                    op=mybir.AluOpType.subtract)

            # ---- stage 2: bounce to DRAM, replicate ----
            nc.sync.dma_start(
                out=bass.AP(tensor=stage_x.tensor, offset=0,
                            ap=[[TK, N], [1, TK]]),
                in_=wx)
            nc.sync.dma_start(
                out=bass.AP(tensor=stage_y.tensor, offset=0,
                            ap=[[TK, N], [1, TK]]),
                in_=wy)
            nc.sync.dma_start(
                out=bass.AP(tensor=stage_r.tensor, offset=0,
                            ap=[[1, N], [1, 1]]),
                in_=rad)

            xj = rep.tile([128, TK], FD)
            yj = rep.tile([128, TK], FD)
            radj = rep.tile([128, 1], FD)
            radi = rep.tile([128, NPAIR], FD)
            for ih in range(2):
                sl = slice(ih * N, (ih + 1) * N)
                nc.sync.dma_start(
                    out=xj[sl, :],
                    in_=bass.AP(tensor=stage_x.tensor, offset=0,
                                ap=[[TK, N], [1, TK]]))
                nc.sync.dma_start(
                    out=yj[sl, :],
                    in_=bass.AP(tensor=stage_y.tensor, offset=0,
                                ap=[[TK, N], [1, TK]]))
                nc.sync.dma_start(
                    out=radj[sl, :],
                    in_=bass.AP(tensor=stage_r.tensor, offset=0,
                                ap=[[1, N], [0, 1]]))
                nc.sync.dma_start(
                    out=radi[sl, :],
                    in_=bass.AP(tensor=stage_r.tensor, offset=HOFF + ih,
                                ap=[[0, N], [2, NPAIR]]))

            wtt = rep.tile([128, NPAIR * T], FD)
            nc.sync.dma_start(out=wtt, in_=wt_in)

            # pd = radi + radj + BUFFER ; invpd = 1/pd
            pd = rep.tile([128, NPAIR], FD)
            nc.vector.tensor_scalar(out=pd, in0=radi, scalar1=radj,
                                    scalar2=float(BUFFER_DIST),
                                    op0=mybir.AluOpType.add,
                                    op1=mybir.AluOpType.add)
            invpd = rep.tile([128, NPAIR], FD)
            nc.vector.reciprocal(out=invpd, in_=pd)

            dminb = acc.tile([128, NPAIR * T], FD)

            def xj_tkl(src):    # [128,(t,k? no t,l)] -> (t,k,l): k bcast
                return bass.AP(tensor=src.tensor, offset=src.offset,
                               ap=[src.ap[0], [K, T], [0, K], [1, K]])

            # ---- stage 3: main loop over i-pairs ----
            for ip in range(NPAIR):
                xi = xiyi.tile([128, TK], FD, tag="xi")
                yi = xiyi.tile([128, TK], FD, tag="yi")
                off = (HOFF + 2 * ip) * TK
                for ih in range(2):
                    sl = slice(ih * N, (ih + 1) * N)
                    nc.sync.dma_start(
                        out=xi[sl, :],
                        in_=bass.AP(tensor=stage_x.tensor,
                                    offset=off + ih * TK,
                                    ap=[[0, N], [1, TK]]))
                    nc.sync.dma_start(
                        out=yi[sl, :],
                        in_=bass.AP(tensor=stage_y.tensor,
                                    offset=off + ih * TK,
                                    ap=[[0, N], [1, TK]]))

                def xi_tkl(src):  # (t,k) -> (t,k,l): l bcast
                    return bass.AP(tensor=src.tensor, offset=src.offset,
                                   ap=[src.ap[0], [K, T], [1, K], [0, K]])

                dx = work.tile([128, TKL], FD, tag="dx")
                dy = work.tile([128, TKL], FD, tag="dy")
                dx3 = dx[:, :].rearrange("p (t k l) -> p t k l", k=K, l=K)
                dy3 = dy[:, :].rearrange("p (t k l) -> p t k l", k=K, l=K)
                nc.vector.tensor_tensor(out=dx3, in0=xi_tkl(xi),
                                        in1=xj_tkl(xj),
                                        op=mybir.AluOpType.subtract)
                nc.vector.tensor_tensor(out=dy3, in0=xi_tkl(yi),
                                        in1=xj_tkl(yj),
                                        op=mybir.AluOpType.subtract)
                sqx = work.tile([128, TKL], FD, tag="sqx")
                sqy = work.tile([128, TKL], FD, tag="sqy")
                nc.scalar.activation(out=sqx, in_=dx,
                                     func=mybir.ActivationFunctionType.Square,
                                     bias=zero128)
                nc.scalar.activation(out=sqy, in_=dy,
                                     func=mybir.ActivationFunctionType.Square,
                                     bias=zero128)
                d2 = work.tile([128, TKL], FD, tag="d2")
                nc.gpsimd.tensor_tensor(out=d2, in0=sqx, in1=sqy,
                                        op=mybir.AluOpType.add)
                d23 = d2[:, :].rearrange("p (t kl) -> p t kl", kl=K * K)
                nc.vector.tensor_reduce(
                    out=dminb[:, ip * T:(ip + 1) * T], in_=d23,
                    axis=mybir.AxisListType.X, op=mybir.AluOpType.min)

            # ---- stage 4: finish ----
            nc.vector.tensor_scalar(out=dminb, in0=dminb, scalar1=0.0,
                                    scalar2=0.0, op0=mybir.AluOpType.max,
                                    op1=mybir.AluOpType.add)
            dist = acc.tile([128, NPAIR * T], FD)
            nc.scalar.activation(out=dist, in_=dminb,
                                 func=mybir.ActivationFunctionType.Sqrt,
                                 bias=zero128)
            # q = dist * invpd  (invpd broadcast over t)
            q = acc.tile([128, NPAIR * T], FD)
            q3 = q[:, :].rearrange("p (i t) -> p i t", t=T)
            d3 = dist[:, :].rearrange("p (i t) -> p i t", t=T)
            nc.vector.tensor_tensor(
                out=q3, in0=d3,
                in1=bass.AP(tensor=invpd.tensor, offset=invpd.offset,
                            ap=[invpd.ap[0], [1, NPAIR], [0, T]]),
                op=mybir.AluOpType.mult)
            # pen_neg = min(q - 1, 0)
            nc.vector.tensor_scalar(out=q, in0=q, scalar1=1.0, scalar2=0.0,
                                    op0=mybir.AluOpType.subtract,
                                    op1=mybir.AluOpType.min)
            # weight (mask * ew / BNT) and reduce
            nc.vector.tensor_tensor(out=q, in0=q, in1=wtt,
                                    op=mybir.AluOpType.mult)
            part = acc.tile([128, 1], FD)
            nc.vector.tensor_reduce(out=part, in_=q,
                                    axis=mybir.AxisListType.X,
                                    op=mybir.AluOpType.add)
            nc.sync.dma_start(out=part_out, in_=part)

    nc.compile()
    return nc


def kernel(Y, length, width):
    Y = np.asarray(Y, np.float32)
    length = np.asarray(length, np.float32)
    width = np.asarray(width, np.float32)

    if "nc" not in _CACHE:
        _CACHE["nc"] = _build()
    nc = _CACHE["nc"]

    frac = (2.0 * np.arange(K, dtype=np.float32) / (K - 1) - 1.0).astype(np.float32)
    ew = DECAY_RATE ** np.arange(T, dtype=np.float32)
    ew = ew / ew.sum()

    in_maps = []
    for c in range(NCORES):
        b, h = divmod(c, 2)
        # rotate agents so i-agents are rows [0,32)
        perm = np.r_[h * 32:(h * 32 + N)] % N
        Yb = Y[b][perm].reshape(N, T * D)
        lb = length[b][perm].reshape(N, 1)
        wb = width[b][perm].reshape(N, 1)
        # weight tile: [128 = (parity, j), NPAIR*T]
        wt = np.zeros((128, NPAIR * T), np.float32)
        for p in range(128):
            ih, j = divmod(p, N)
            for ip in range(NPAIR):
                i_loc = 2 * ip + ih          # row in permuted agent axis
                if i_loc != j:               # mask diagonal
                    wt[p, ip * T:(ip + 1) * T] = ew / (B * N * T)
        in_maps.append({
            "y_in": Yb, "len_in": lb, "wid_in": wb,
            "frac_in": frac, "wt_in": wt,
        })

    global _LAST_INMAPS
    _LAST_INMAPS = in_maps
    res = bass_utils.run_bass_kernel_spmd(nc, in_maps,
                                          core_ids=list(range(NCORES)))
    total = 0.0
    for c in range(NCORES):
        total += float(res.results[c]["part_out"].astype(np.float64).sum())
    return np.float32(-total)


# revision 8
# speedup vs baseline: 1.0090x; 1.0090x over previous
"""AgentCollisionLoss Trainium2 kernel.

Sharding: 8 cores = B(4) x i-half(2). Core c handles b = c//2 and the 32
i-agents [h*32, h*32+32) vs all 64 j-agents over all T=80 steps.

On-device per core:
  world disk coords wx/wy for all 64 agents  (partitions = j)
  loop over 16 i-pairs: partitions p = (i-parity)*64 + j, free = (t,k,l)
    dx = xi - xj, dy = yi - yj           (DVE tensor_tensor, fp32)
    sqx = dx^2, sqy = dy^2               (ACT Square)
    d2 = sqx + sqy                       (GPSIMD scalar_tensor_tensor)
    dmin2 = min over (k,l)               (DVE tensor_reduce)
  clamp, sqrt (ACT), q = dmin/pd, pen = min(q-1, 0), weight+sum
Host: slice inputs per core, sum the 8x[128] partials, negate.
"""

import numpy as np

import concourse.bass as bass
import concourse.bacc as bacc
import concourse.tile as tile
import concourse.mybir as mybir
from concourse import bass_utils

B, N, T, D = 4, 64, 80, 6
K = 5
NCORES = 8
BUFFER_DIST = 0.2
DECAY_RATE = 0.9
TK = T * K          # 400
TKL = T * K * K     # 2000
NPAIR = 16          # i-pairs per core (32 i-agents / 2)
FD = mybir.dt.float32

_CACHE = {}
_LAST_INMAPS = None


def _build():
    nc = bacc.Bacc("TRN2", target_bir_lowering=False, debug=False,
                   num_devices=NCORES)

    y_in = nc.dram_tensor("y_in", [N, T * D], FD, kind="ExternalInput").ap()
    len_in = nc.dram_tensor("len_in", [N, 1], FD, kind="ExternalInput").ap()
    wid_in = nc.dram_tensor("wid_in", [N, 1], FD, kind="ExternalInput").ap()
    frac_in = nc.dram_tensor("frac_in", [K], FD, kind="ExternalInput").ap()
    wt_in = nc.dram_tensor("wt_in", [128, NPAIR * T], FD,
                           kind="ExternalInput").ap()
    part_out = nc.dram_tensor("part_out", [128, 1], FD,
                              kind="ExternalOutput").ap()

    stage_x = nc.dram_tensor("stage_x", [N * TK], FD, kind="Internal").ap()
    stage_y = nc.dram_tensor("stage_y", [N * TK], FD, kind="Internal").ap()
    stage_r = nc.dram_tensor("stage_r", [N], FD, kind="Internal").ap()

    # the per-core i-half offset is baked per-core via h below; we build one
    # program (SPMD) so instead we bake h into the *host-provided* Wt and by
    # providing per-core inputs; the i-slice offset must be identical across
    # cores, so the host rotates the agent axis per core such that the
    # i-agents are always rows [0, 32).
    HOFF = 0

    with tile.TileContext(nc) as tc:
        with (
            tc.tile_pool(name="prep", bufs=1) as prep,
            tc.tile_pool(name="rep", bufs=1) as rep,
            tc.tile_pool(name="xiyi", bufs=4) as xiyi,
            tc.tile_pool(name="work", bufs=2) as work,
            tc.tile_pool(name="acc", bufs=1) as acc,
        ):
            # ---- stage 1: per-agent prep (partitions = j, 64) ----
            ytile = prep.tile([N, T * D], FD)
            nc.sync.dma_start(out=ytile, in_=y_in)
            def ycol(dcol):
                return bass.AP(tensor=ytile.tensor,
                               offset=ytile.offset + dcol,
                               ap=[ytile.ap[0], [D, T]])
            x_ap = ycol(0)
            ypos_ap = ycol(1)
            yaw_ap = ycol(4)

            ltile = prep.tile([N, 1], FD)
            wtile = prep.tile([N, 1], FD)
            nc.sync.dma_start(out=ltile, in_=len_in)
            nc.sync.dma_start(out=wtile, in_=wid_in)
            fr = prep.tile([N, K], FD)
            nc.sync.dma_start(
                out=fr,
                in_=bass.AP(tensor=frac_in.tensor, offset=0,
                            ap=[[0, N], [1, K]]))

            zero128 = prep.tile([128, 1], FD)
            nc.vector.memset(zero128, 0.0)
            pi2 = prep.tile([N, 1], FD)
            nc.vector.memset(pi2, float(np.pi / 2))
            cosT = prep.tile([N, T], FD)
            sinT = prep.tile([N, T], FD)
            nc.scalar.activation(out=cosT, in_=yaw_ap,
                                 func=mybir.ActivationFunctionType.Sin,
                                 bias=pi2, scale=1.0)
            nc.scalar.activation(out=sinT, in_=yaw_ap,
                                 func=mybir.ActivationFunctionType.Sin,
                                 bias=zero128[:N, :], scale=1.0)

            rad = prep.tile([N, 1], FD)
            nc.vector.tensor_scalar(out=rad, in0=wtile, scalar1=0.5,
                                    scalar2=0.0, op0=mybir.AluOpType.mult,
                                    op1=mybir.AluOpType.add)
            # cmax = l/2 - rad ; cmin = -cmax ; cent = cmin + (cmax-cmin)*frac
            cmax = prep.tile([N, 1], FD)
            nc.vector.scalar_tensor_tensor(out=cmax, in0=ltile, scalar=0.5,
                                           in1=rad,
                                           op0=mybir.AluOpType.mult,
                                           op1=mybir.AluOpType.subtract)
            # cent[j,l] = cmax * f2[l]   (host provides f2 = 2*frac-1)
            cent = prep.tile([N, K], FD)
            nc.vector.tensor_scalar(out=cent, in0=fr, scalar1=cmax,
                                    scalar2=0.0,
                                    op0=mybir.AluOpType.mult,
                                    op1=mybir.AluOpType.add)

            wx = prep.tile([N, TK], FD)
            wy = prep.tile([N, TK], FD)
            # wx = cent[j,l]*cos[j,t] + x[j,t]
            tmp = prep.tile([N, TK], FD)

            def bc_tl(src_t):   # [N,T] -> (t,l) view
                return bass.AP(tensor=src_t.tensor, offset=src_t.offset,
                               ap=[src_t.ap[0], [src_t.ap[-1][0], T], [0, K]])

            def bc_lt(src_l):   # [N,K] -> (t,l) view
                return bass.AP(tensor=src_l.tensor, offset=src_l.offset,
                               ap=[src_l.ap[0], [0, T], [src_l.ap[-1][0], K]])

            wx3 = wx[:, :].rearrange("p (t l) -> p t l", l=K)
            wy3 = wy[:, :].rearrange("p (t l) -> p t l", l=K)
            tmp3 = tmp[:, :].rearrange("p (t l) -> p t l", l=K)

            nc.vector.tensor_tensor(out=tmp3, in0=bc_tl(cosT), in1=bc_lt(cent),
                                    op=mybir.AluOpType.mult)
            nc.vector.tensor_tensor(out=wx3, in0=tmp3, in1=bc_tl(x_ap),
                                    op=mybir.AluOpType.add)
            nc.vector.tensor_tensor(out=tmp3, in0=bc_tl(sinT), in1=bc_lt(cent),
                                    op=mybir.AluOpType.mult)
            nc.vector.tensor_tensor(out=wy3, in0=bc_tl(ypos_ap), in1=tmp3,
                                    op=mybir.AluOpType.subtract)

            # ---- stage 2: bounce to DRAM, replicate ----
            nc.sync.dma_start(
                out=bass.AP(tensor=stage_x.tensor, offset=0,
                            ap=[[TK, N], [1, TK]]),
                in_=wx)
            nc.sync.dma_start(
                out=bass.AP(tensor=stage_y.tensor, offset=0,
                            ap=[[TK, N], [1, TK]]),
                in_=wy)
            nc.sync.dma_start(
                out=bass.AP(tensor=stage_r.tensor, offset=0,
                            ap=[[1, N], [1, 1]]),
                in_=rad)

            xj = rep.tile([128, TK], FD)
            yj = rep.tile([128, TK], FD)
            radj = rep.tile([128, 1], FD)
            radi = rep.tile([128, NPAIR], FD)
            for ih in range(2):
                sl = slice(ih * N, (ih + 1) * N)
                nc.sync.dma_start(
                    out=xj[sl, :],
                    in_=bass.AP(tensor=stage_x.tensor, offset=0,
                                ap=[[TK, N], [1, TK]]))
                nc.sync.dma_start(
                    out=yj[sl, :],
                    in_=bass.AP(tensor=stage_y.tensor, offset=0,
                                ap=[[TK, N], [1, TK]]))
                nc.sync.dma_start(
                    out=radj[sl, :],
                    in_=bass.AP(tensor=stage_r.tensor, offset=0,
                                ap=[[1, N], [0, 1]]))
                nc.sync.dma_start(
                    out=radi[sl, :],
                    in_=bass.AP(tensor=stage_r.tensor, offset=HOFF + ih,
                                ap=[[0, N], [2, NPAIR]]))

            wtt = rep.tile([128, NPAIR * T], FD)
            nc.sync.dma_start(out=wtt, in_=wt_in)

            # pd = radi + radj + BUFFER ; invpd = 1/pd
            pd = rep.tile([128, NPAIR], FD)
            nc.vector.tensor_scalar(out=pd, in0=radi, scalar1=radj,
                                    scalar2=float(BUFFER_DIST),
                                    op0=mybir.AluOpType.add,
                                    op1=mybir.AluOpType.add)
            invpd = rep.tile([128, NPAIR], FD)
            nc.vector.reciprocal(out=invpd, in_=pd)

            dminb = acc.tile([128, NPAIR * T], FD)

            def xj_tkl(src):    # [128,(t,k? no t,l)] -> (t,k,l): k bcast
                return bass.AP(tensor=src.tensor, offset=src.offset,
                               ap=[src.ap[0], [K, T], [0, K], [1, K]])

            # ---- stage 3: main loop over i-pairs ----
            for ip in range(NPAIR):
                xi = xiyi.tile([128, TK], FD, tag="xi")
                yi = xiyi.tile([128, TK], FD, tag="yi")
                off = (HOFF + 2 * ip) * TK
                for ih in range(2):
                    sl = slice(ih * N, (ih + 1) * N)
                    nc.sync.dma_start(
                        out=xi[sl, :],
                        in_=bass.AP(tensor=stage_x.tensor,
                                    offset=off + ih * TK,
                                    ap=[[0, N], [1, TK]]))
                    nc.sync.dma_start(
                        out=yi[sl, :],
                        in_=bass.AP(tensor=stage_y.tensor,
                                    offset=off + ih * TK,
                                    ap=[[0, N], [1, TK]]))

                def xi_tkl(src):  # (t,k) -> (t,k,l): l bcast
                    return bass.AP(tensor=src.tensor, offset=src.offset,
                                   ap=[src.ap[0], [K, T], [1, K], [0, K]])

                dx = work.tile([128, TKL], FD, tag="dx")
                dy = work.tile([128, TKL], FD, tag="dy")
                dx3 = dx[:, :].rearrange("p (t k l) -> p t k l", k=K, l=K)
                dy3 = dy[:, :].rearrange("p (t k l) -> p t k l", k=K, l=K)
                nc.vector.tensor_tensor(out=dx3, in0=xi_tkl(xi),
                                        in1=xj_tkl(xj),
                                        op=mybir.AluOpType.subtract)
                nc.vector.tensor_tensor(out=dy3, in0=xi_tkl(yi),
                                        in1=xj_tkl(yj),
                                        op=mybir.AluOpType.subtract)
                sqx = work.tile([128, TKL], FD, tag="sqx")
                sqy = work.tile([128, TKL], FD, tag="sqy")
                nc.scalar.activation(out=sqx, in_=dx,
                                     func=mybir.ActivationFunctionType.Square,
                                     bias=zero128)
                nc.scalar.activation(out=sqy, in_=dy,
                                     func=mybir.ActivationFunctionType.Square,
                                     bias=zero128)
                d2 = work.tile([128, TKL], FD, tag="d2")
                nc.gpsimd.tensor_tensor(out=d2, in0=sqx, in1=sqy,
                                        op=mybir.AluOpType.add)
                d23 = d2[:, :].rearrange("p (t kl) -> p t kl", kl=K * K)
                nc.vector.tensor_reduce(
                    out=dminb[:, ip * T:(ip + 1) * T], in_=d23,
                    axis=mybir.AxisListType.X, op=mybir.AluOpType.min)

            # ---- stage 4: finish ----
            nc.vector.tensor_scalar(out=dminb, in0=dminb, scalar1=0.0,
                                    scalar2=0.0, op0=mybir.AluOpType.max,
                                    op1=mybir.AluOpType.add)
            dist = acc.tile([128, NPAIR * T], FD)
            nc.scalar.activation(out=dist, in_=dminb,
                                 func=mybir.ActivationFunctionType.Sqrt,
                                 bias=zero128)
            # q = dist * invpd  (invpd broadcast over t)
            q = acc.tile([128, NPAIR * T], FD)
            q3 = q[:, :].rearrange("p (i t) -> p i t", t=T)
            d3 = dist[:, :].rearrange("p (i t) -> p i t", t=T)
            nc.vector.tensor_tensor(
                out=q3, in0=d3,
                in1=bass.AP(tensor=invpd.tensor, offset=invpd.offset,
                            ap=[invpd.ap[0], [1, NPAIR], [0, T]]),
                op=mybir.AluOpType.mult)
            # pen_neg = min(q - 1, 0)
            nc.vector.tensor_scalar(out=q, in0=q, scalar1=1.0, scalar2=0.0,
                                    op0=mybir.AluOpType.subtract,
                                    op1=mybir.AluOpType.min)
            # weight (mask * ew / BNT) and reduce
            nc.vector.tensor_tensor(out=q, in0=q, in1=wtt,
                                    op=mybir.AluOpType.mult)
            part = acc.tile([128, 1], FD)
            nc.vector.tensor_reduce(out=part, in_=q,
                                    axis=mybir.AxisListType.X,
                                    op=mybir.AluOpType.add)
            nc.sync.dma_start(out=part_out, in_=part)

    nc.compile()
    return nc


def kernel(Y, length, width):
    Y = np.asarray(Y, np.float32)
    length = np.asarray(length, np.float32)
    width = np.asarray(width, np.float32)

    if "nc" not in _CACHE:
        _CACHE["nc"] = _build()
    nc = _CACHE["nc"]

    frac = (2.0 * np.arange(K, dtype=np.float32) / (K - 1) - 1.0).astype(np.float32)
    ew = DECAY_RATE ** np.arange(T, dtype=np.float32)
    ew = ew / ew.sum()

    in_maps = []
    for c in range(NCORES):
        b, h = divmod(c, 2)
        # rotate agents so i-agents are rows [0,32)
        perm = np.r_[h * 32:(h * 32 + N)] % N
        Yb = Y[b][perm].reshape(N, T * D)
        lb = length[b][perm].reshape(N, 1)
        wb = width[b][perm].reshape(N, 1)
        # weight tile: [128 = (parity, j), NPAIR*T]
        wt = np.zeros((128, NPAIR * T), np.float32)
        for p in range(128):
            ih, j = divmod(p, N)
            for ip in range(NPAIR):
                i_loc = 2 * ip + ih          # row in permuted agent axis
                if i_loc != j:               # mask diagonal
                    wt[p, ip * T:(ip + 1) * T] = ew / (B * N * T)
        in_maps.append({
            "y_in": Yb, "len_in": lb, "wid_in": wb,
            "frac_in": frac, "wt_in": wt,
        })

    global _LAST_INMAPS
    _LAST_INMAPS = in_maps
    res = bass_utils.run_bass_kernel_spmd(nc, in_maps,
                                          core_ids=list(range(NCORES)))
    total = 0.0
    for c in range(NCORES):
        total += float(res.results[c]["part_out"].astype(np.float64).sum())
    return np.float32(-total)


# revision 9
# speedup vs baseline: 1.0930x; 1.0833x over previous
"""AgentCollisionLoss Trainium2 kernel.

Sharding: 8 cores = B(4) x i-half(2). Core c handles b = c//2 and the 32
i-agents [h*32, h*32+32) vs all 64 j-agents over all T=80 steps.

On-device per core:
  world disk coords wx/wy for all 64 agents  (partitions = j)
  loop over 16 i-pairs: partitions p = (i-parity)*64 + j, free = (t,k,l)
    dx = xi - xj, dy = yi - yj           (DVE tensor_tensor, fp32)
    sqx = dx^2, sqy = dy^2               (ACT Square)
    d2 = sqx + sqy                       (GPSIMD scalar_tensor_tensor)
    dmin2 = min over (k,l)               (DVE tensor_reduce)
  clamp, sqrt (ACT), q = dmin/pd, pen = min(q-1, 0), weight+sum
Host: slice inputs per core, sum the 8x[128] partials, negate.
"""

import numpy as np

import concourse.bass as bass
import concourse.bacc as bacc
import concourse.tile as tile
import concourse.mybir as mybir
from concourse import bass_utils

B, N, T, D = 4, 64, 80, 6
K = 5
NCORES = 8
BUFFER_DIST = 0.2
DECAY_RATE = 0.9
TK = T * K          # 400
TKL = T * K * K     # 2000
NPAIR = 16          # i-pairs per core (32 i-agents / 2)
FD = mybir.dt.float32

_CACHE = {}
_LAST_INMAPS = None


def _build():
    nc = bacc.Bacc("TRN2", target_bir_lowering=False, debug=False,
                   num_devices=NCORES)

    y_in = nc.dram_tensor("y_in", [N, T * D], FD, kind="ExternalInput").ap()
    len_in = nc.dram_tensor("len_in", [N, 1], FD, kind="ExternalInput").ap()
    wid_in = nc.dram_tensor("wid_in", [N, 1], FD, kind="ExternalInput").ap()
    frac_in = nc.dram_tensor("frac_in", [K], FD, kind="ExternalInput").ap()
    wt_in = nc.dram_tensor("wt_in", [128, NPAIR * T], FD,
                           kind="ExternalInput").ap()
    part_out = nc.dram_tensor("part_out", [128, 1], FD,
                              kind="ExternalOutput").ap()

    stage_x = nc.dram_tensor("stage_x", [N * TK], FD, kind="Internal").ap()
    stage_y = nc.dram_tensor("stage_y", [N * TK], FD, kind="Internal").ap()
    stage_r = nc.dram_tensor("stage_r", [N], FD, kind="Internal").ap()

    # the per-core i-half offset is baked per-core via h below; we build one
    # program (SPMD) so instead we bake h into the *host-provided* Wt and by
    # providing per-core inputs; the i-slice offset must be identical across
    # cores, so the host rotates the agent axis per core such that the
    # i-agents are always rows [0, 32).
    HOFF = 0

    with tile.TileContext(nc) as tc:
        with (
            tc.tile_pool(name="prep", bufs=1) as prep,
            tc.tile_pool(name="rep", bufs=1) as rep,
            tc.tile_pool(name="xiyi", bufs=4) as xiyi,
            tc.tile_pool(name="work", bufs=2) as work,
            tc.tile_pool(name="acc", bufs=1) as acc,
        ):
            # ---- stage 1: per-agent prep (partitions = j, 64) ----
            ytile = prep.tile([N, T * D], FD)
            nc.sync.dma_start(out=ytile, in_=y_in)
            def ycol(dcol):
                return bass.AP(tensor=ytile.tensor,
                               offset=ytile.offset + dcol,
                               ap=[ytile.ap[0], [D, T]])
            x_ap = ycol(0)
            ypos_ap = ycol(1)
            yaw_ap = ycol(4)

            ltile = prep.tile([N, 1], FD)
            wtile = prep.tile([N, 1], FD)
            nc.sync.dma_start(out=ltile, in_=len_in)
            nc.sync.dma_start(out=wtile, in_=wid_in)
            fr = prep.tile([N, K], FD)
            nc.sync.dma_start(
                out=fr,
                in_=bass.AP(tensor=frac_in.tensor, offset=0,
                            ap=[[0, N], [1, K]]))

            zero128 = prep.tile([128, 1], FD)
            nc.vector.memset(zero128, 0.0)
            pi2 = prep.tile([N, 1], FD)
            nc.vector.memset(pi2, float(np.pi / 2))
            cosT = prep.tile([N, T], FD)
            sinT = prep.tile([N, T], FD)
            nc.scalar.activation(out=cosT, in_=yaw_ap,
                                 func=mybir.ActivationFunctionType.Sin,
                                 bias=pi2, scale=1.0)
            nc.scalar.activation(out=sinT, in_=yaw_ap,
                                 func=mybir.ActivationFunctionType.Sin,
                                 bias=zero128[:N, :], scale=1.0)

            rad = prep.tile([N, 1], FD)
            nc.vector.tensor_scalar(out=rad, in0=wtile, scalar1=0.5,
                                    scalar2=0.0, op0=mybir.AluOpType.mult,
                                    op1=mybir.AluOpType.add)
            # cmax = l/2 - rad ; cmin = -cmax ; cent = cmin + (cmax-cmin)*frac
            cmax = prep.tile([N, 1], FD)
            nc.vector.scalar_tensor_tensor(out=cmax, in0=ltile, scalar=0.5,
                                           in1=rad,
                                           op0=mybir.AluOpType.mult,
                                           op1=mybir.AluOpType.subtract)
            # cent[j,l] = cmax * f2[l]   (host provides f2 = 2*frac-1)
            cent = prep.tile([N, K], FD)
            nc.vector.tensor_scalar(out=cent, in0=fr, scalar1=cmax,
                                    scalar2=0.0,
                                    op0=mybir.AluOpType.mult,
                                    op1=mybir.AluOpType.add)

            wx = prep.tile([N, TK], FD)
            wy = prep.tile([N, TK], FD)
            # wx = cent[j,l]*cos[j,t] + x[j,t]
            tmp = prep.tile([N, TK], FD)

            def bc_tl(src_t):   # [N,T] -> (t,l) view
                return bass.AP(tensor=src_t.tensor, offset=src_t.offset,
                               ap=[src_t.ap[0], [src_t.ap[-1][0], T], [0, K]])

            def bc_lt(src_l):   # [N,K] -> (t,l) view
                return bass.AP(tensor=src_l.tensor, offset=src_l.offset,
                               ap=[src_l.ap[0], [0, T], [src_l.ap[-1][0], K]])

            wx3 = wx[:, :].rearrange("p (t l) -> p t l", l=K)
            wy3 = wy[:, :].rearrange("p (t l) -> p t l", l=K)
            tmp3 = tmp[:, :].rearrange("p (t l) -> p t l", l=K)

            nc.vector.tensor_tensor(out=tmp3, in0=bc_tl(cosT), in1=bc_lt(cent),
                                    op=mybir.AluOpType.mult)
            nc.vector.tensor_tensor(out=wx3, in0=tmp3, in1=bc_tl(x_ap),
                                    op=mybir.AluOpType.add)
            nc.vector.tensor_tensor(out=tmp3, in0=bc_tl(sinT), in1=bc_lt(cent),
                                    op=mybir.AluOpType.mult)
            nc.vector.tensor_tensor(out=wy3, in0=bc_tl(ypos_ap), in1=tmp3,
                                    op=mybir.AluOpType.subtract)

            # ---- stage 2: bounce to DRAM, replicate ----
            nc.sync.dma_start(
                out=bass.AP(tensor=stage_x.tensor, offset=0,
                            ap=[[TK, N], [1, TK]]),
                in_=wx)
            nc.sync.dma_start(
                out=bass.AP(tensor=stage_y.tensor, offset=0,
                            ap=[[TK, N], [1, TK]]),
                in_=wy)
            nc.sync.dma_start(
                out=bass.AP(tensor=stage_r.tensor, offset=0,
                            ap=[[1, N], [1, 1]]),
                in_=rad)

            xj = rep.tile([128, TK], FD)
            yj = rep.tile([128, TK], FD)
            radj = rep.tile([128, 1], FD)
            radi = rep.tile([128, NPAIR], FD)
            for ih in range(2):
                sl = slice(ih * N, (ih + 1) * N)
                nc.sync.dma_start(
                    out=xj[sl, :],
                    in_=bass.AP(tensor=stage_x.tensor, offset=0,
                                ap=[[TK, N], [1, TK]]))
                nc.sync.dma_start(
                    out=yj[sl, :],
                    in_=bass.AP(tensor=stage_y.tensor, offset=0,
                                ap=[[TK, N], [1, TK]]))
                nc.sync.dma_start(
                    out=radj[sl, :],
                    in_=bass.AP(tensor=stage_r.tensor, offset=0,
                                ap=[[1, N], [0, 1]]))
                nc.sync.dma_start(
                    out=radi[sl, :],
                    in_=bass.AP(tensor=stage_r.tensor, offset=HOFF + ih,
                                ap=[[0, N], [2, NPAIR]]))

            wtt = rep.tile([128, NPAIR * T], FD)
            nc.sync.dma_start(out=wtt, in_=wt_in)

            # pd = radi + radj + BUFFER ; invpd = 1/pd
            pd = rep.tile([128, NPAIR], FD)
            nc.vector.tensor_scalar(out=pd, in0=radi, scalar1=radj,
                                    scalar2=float(BUFFER_DIST),
                                    op0=mybir.AluOpType.add,
                                    op1=mybir.AluOpType.add)
            invpd = rep.tile([128, NPAIR], FD)
            nc.vector.reciprocal(out=invpd, in_=pd)

            dminb = acc.tile([128, NPAIR * T], FD)

            def xj_tkl(src):    # [128,(t,k? no t,l)] -> (t,k,l): k bcast
                return bass.AP(tensor=src.tensor, offset=src.offset,
                               ap=[src.ap[0], [K, T], [0, K], [1, K]])

            # ---- stage 3: main loop over i-pairs ----
            for ip in range(NPAIR):
                xi = xiyi.tile([128, TK], FD, tag="xi")
                yi = xiyi.tile([128, TK], FD, tag="yi")
                off = (HOFF + 2 * ip) * TK
                for ih in range(2):
                    sl = slice(ih * N, (ih + 1) * N)
                    nc.sync.dma_start(
                        out=xi[sl, :],
                        in_=bass.AP(tensor=stage_x.tensor,
                                    offset=off + ih * TK,
                                    ap=[[0, N], [1, TK]]))
                    nc.sync.dma_start(
                        out=yi[sl, :],
                        in_=bass.AP(tensor=stage_y.tensor,
                                    offset=off + ih * TK,
                                    ap=[[0, N], [1, TK]]))

                def xi_tkl(src):  # (t,k) -> (t,k,l): l bcast
                    return bass.AP(tensor=src.tensor, offset=src.offset,
                                   ap=[src.ap[0], [K, T], [1, K], [0, K]])

                dx = work.tile([128, TKL], FD, tag="dx")
                dy = work.tile([128, TKL], FD, tag="dy")
                dx3 = dx[:, :].rearrange("p (t k l) -> p t k l", k=K, l=K)
                dy3 = dy[:, :].rearrange("p (t k l) -> p t k l", k=K, l=K)
                nc.vector.tensor_tensor(out=dx3, in0=xi_tkl(xi),
                                        in1=xj_tkl(xj),
                                        op=mybir.AluOpType.subtract)
                nc.vector.tensor_tensor(out=dy3, in0=xi_tkl(yi),
                                        in1=xj_tkl(yj),
                                        op=mybir.AluOpType.subtract)
                sqx = work.tile([128, TKL], FD, tag="sqx")
                sqy = work.tile([128, TKL], FD, tag="sqy")
                nc.scalar.activation(out=sqx, in_=dx,
                                     func=mybir.ActivationFunctionType.Square,
                                     bias=zero128)
                nc.scalar.activation(out=sqy, in_=dy,
                                     func=mybir.ActivationFunctionType.Square,
                                     bias=zero128)
                d2 = work.tile([128, TKL], FD, tag="d2")
                nc.vector.scalar_tensor_tensor(out=d2, in0=sqx,
                                               scalar=1.0, in1=sqy,
                                               op0=mybir.AluOpType.mult,
                                               op1=mybir.AluOpType.add)
                d23 = d2[:, :].rearrange("p (t kl) -> p t kl", kl=K * K)
                nc.vector.tensor_reduce(
                    out=dminb[:, ip * T:(ip + 1) * T], in_=d23,
                    axis=mybir.AxisListType.X, op=mybir.AluOpType.min)

            # ---- stage 4: finish ----
            nc.vector.tensor_scalar(out=dminb, in0=dminb, scalar1=0.0,
                                    scalar2=0.0, op0=mybir.AluOpType.max,
                                    op1=mybir.AluOpType.add)
            dist = acc.tile([128, NPAIR * T], FD)
            nc.scalar.activation(out=dist, in_=dminb,
                                 func=mybir.ActivationFunctionType.Sqrt,
                                 bias=zero128)
            # q = dist * invpd  (invpd broadcast over t)
            q = acc.tile([128, NPAIR * T], FD)
            q3 = q[:, :].rearrange("p (i t) -> p i t", t=T)
            d3 = dist[:, :].rearrange("p (i t) -> p i t", t=T)
            nc.vector.tensor_tensor(
                out=q3, in0=d3,
                in1=bass.AP(tensor=invpd.tensor, offset=invpd.offset,
                            ap=[invpd.ap[0], [1, NPAIR], [0, T]]),
                op=mybir.AluOpType.mult)
            # pen_neg = min(q - 1, 0)
            nc.vector.tensor_scalar(out=q, in0=q, scalar1=1.0, scalar2=0.0,
                                    op0=mybir.AluOpType.subtract,
                                    op1=mybir.AluOpType.min)
            # weight (mask * ew / BNT) and reduce
            nc.vector.tensor_tensor(out=q, in0=q, in1=wtt,
                                    op=mybir.AluOpType.mult)
            part = acc.tile([128, 1], FD)
            nc.vector.tensor_reduce(out=part, in_=q,
                                    axis=mybir.AxisListType.X,
                                    op=mybir.AluOpType.add)
            nc.sync.dma_start(out=part_out, in_=part)

    nc.compile()
    return nc


def kernel(Y, length, width):
    Y = np.asarray(Y, np.float32)
    length = np.asarray(length, np.float32)
    width = np.asarray(width, np.float32)

    if "nc" not in _CACHE:
        _CACHE["nc"] = _build()
    nc = _CACHE["nc"]

    frac = (2.0 * np.arange(K, dtype=np.float32) / (K - 1) - 1.0).astype(np.float32)
    ew = DECAY_RATE ** np.arange(T, dtype=np.float32)
    ew = ew / ew.sum()

    in_maps = []
    for c in range(NCORES):
        b, h = divmod(c, 2)
        # rotate agents so i-agents are rows [0,32)
        perm = np.r_[h * 32:(h * 32 + N)] % N
        Yb = Y[b][perm].reshape(N, T * D)
        lb = length[b][perm].reshape(N, 1)
        wb = width[b][perm].reshape(N, 1)
        # weight tile: [128 = (parity, j), NPAIR*T]
        wt = np.zeros((128, NPAIR * T), np.float32)
        for p in range(128):
            ih, j = divmod(p, N)
            for ip in range(NPAIR):
                i_loc = 2 * ip + ih          # row in permuted agent axis
                if i_loc != j:               # mask diagonal
                    wt[p, ip * T:(ip + 1) * T] = ew / (B * N * T)
        in_maps.append({
            "y_in": Yb, "len_in": lb, "wid_in": wb,
            "frac_in": frac, "wt_in": wt,
        })

    global _LAST_INMAPS
    _LAST_INMAPS = in_maps
    res = bass_utils.run_bass_kernel_spmd(nc, in_maps,
                                          core_ids=list(range(NCORES)))
    total = 0.0
    for c in range(NCORES):
        total += float(res.results[c]["part_out"].astype(np.float64).sum())
    return np.float32(-total)


# revision 10
# speedup vs baseline: 1.1811x; 1.0806x over previous
"""AgentCollisionLoss Trainium2 kernel.

Sharding: 8 cores = B(4) x i-half(2). Core c handles b = c//2 and the 32
i-agents [h*32, h*32+32) vs all 64 j-agents over all T=80 steps.

On-device per core:
  world disk coords wx/wy for all 64 agents  (partitions = j)
  loop over 16 i-pairs: partitions p = (i-parity)*64 + j, free = (t,k,l)
    dx = xi - xj, dy = yi - yj           (DVE tensor_tensor, fp32)
    sqx = dx^2, sqy = dy^2               (ACT Square)
    d2 = sqx + sqy                       (GPSIMD scalar_tensor_tensor)
    dmin2 = min over (k,l)               (DVE tensor_reduce)
  clamp, sqrt (ACT), q = dmin/pd, pen = min(q-1, 0), weight+sum
Host: slice inputs per core, sum the 8x[128] partials, negate.
"""

import numpy as np

import concourse.bass as bass
import concourse.bacc as bacc
import concourse.tile as tile
import concourse.mybir as mybir
from concourse import bass_utils

B, N, T, D = 4, 64, 80, 6
K = 5
NCORES = 8
BUFFER_DIST = 0.2
DECAY_RATE = 0.9
TK = T * K          # 400
TKL = T * K * K     # 2000
NPAIR = 16          # i-pairs per core (32 i-agents / 2)
FD = mybir.dt.float32

_CACHE = {}
_LAST_INMAPS = None


def _build():
    nc = bacc.Bacc("TRN2", target_bir_lowering=False, debug=False,
                   num_devices=NCORES)

    y_in = nc.dram_tensor("y_in", [N, T * D], FD, kind="ExternalInput").ap()
    len_in = nc.dram_tensor("len_in", [N, 1], FD, kind="ExternalInput").ap()
    wid_in = nc.dram_tensor("wid_in", [N, 1], FD, kind="ExternalInput").ap()
    frac_in = nc.dram_tensor("frac_in", [K], FD, kind="ExternalInput").ap()
    wt_in = nc.dram_tensor("wt_in", [128, NPAIR * T], FD,
                           kind="ExternalInput").ap()
    part_out = nc.dram_tensor("part_out", [128, 1], FD,
                              kind="ExternalOutput").ap()

    stage_x = nc.dram_tensor("stage_x", [N * TK], FD, kind="Internal").ap()
    stage_y = nc.dram_tensor("stage_y", [N * TK], FD, kind="Internal").ap()
    stage_r = nc.dram_tensor("stage_r", [N], FD, kind="Internal").ap()

    # the per-core i-half offset is baked per-core via h below; we build one
    # program (SPMD) so instead we bake h into the *host-provided* Wt and by
    # providing per-core inputs; the i-slice offset must be identical across
    # cores, so the host rotates the agent axis per core such that the
    # i-agents are always rows [0, 32).
    HOFF = 0

    with tile.TileContext(nc) as tc:
        with (
            tc.tile_pool(name="prep", bufs=1) as prep,
            tc.tile_pool(name="rep", bufs=1) as rep,
            tc.tile_pool(name="xiyi", bufs=2) as xiyi,
            tc.tile_pool(name="work", bufs=1) as work,
            tc.tile_pool(name="acc", bufs=1) as acc,
        ):
            # ---- stage 1: per-agent prep (partitions = j, 64) ----
            ytile = prep.tile([N, T * D], FD)
            nc.sync.dma_start(out=ytile, in_=y_in)
            def ycol(dcol):
                return bass.AP(tensor=ytile.tensor,
                               offset=ytile.offset + dcol,
                               ap=[ytile.ap[0], [D, T]])
            x_ap = ycol(0)
            ypos_ap = ycol(1)
            yaw_ap = ycol(4)

            ltile = prep.tile([N, 1], FD)
            wtile = prep.tile([N, 1], FD)
            nc.sync.dma_start(out=ltile, in_=len_in)
            nc.sync.dma_start(out=wtile, in_=wid_in)
            fr = prep.tile([N, K], FD)
            nc.sync.dma_start(
                out=fr,
                in_=bass.AP(tensor=frac_in.tensor, offset=0,
                            ap=[[0, N], [1, K]]))

            zero128 = prep.tile([128, 1], FD)
            nc.vector.memset(zero128, 0.0)
            pi2 = prep.tile([N, 1], FD)
            nc.vector.memset(pi2, float(np.pi / 2))
            cosT = prep.tile([N, T], FD)
            sinT = prep.tile([N, T], FD)
            nc.scalar.activation(out=cosT, in_=yaw_ap,
                                 func=mybir.ActivationFunctionType.Sin,
                                 bias=pi2, scale=1.0)
            nc.scalar.activation(out=sinT, in_=yaw_ap,
                                 func=mybir.ActivationFunctionType.Sin,
                                 bias=zero128[:N, :], scale=1.0)

            rad = prep.tile([N, 1], FD)
            nc.vector.tensor_scalar(out=rad, in0=wtile, scalar1=0.5,
                                    scalar2=0.0, op0=mybir.AluOpType.mult,
                                    op1=mybir.AluOpType.add)
            # cmax = l/2 - rad ; cmin = -cmax ; cent = cmin + (cmax-cmin)*frac
            cmax = prep.tile([N, 1], FD)
            nc.vector.scalar_tensor_tensor(out=cmax, in0=ltile, scalar=0.5,
                                           in1=rad,
                                           op0=mybir.AluOpType.mult,
                                           op1=mybir.AluOpType.subtract)
            # cent[j,l] = cmax * f2[l]   (host provides f2 = 2*frac-1)
            cent = prep.tile([N, K], FD)
            nc.vector.tensor_scalar(out=cent, in0=fr, scalar1=cmax,
                                    scalar2=0.0,
                                    op0=mybir.AluOpType.mult,
                                    op1=mybir.AluOpType.add)

            wx = prep.tile([N, TK], FD)
            wy = prep.tile([N, TK], FD)
            # wx = cent[j,l]*cos[j,t] + x[j,t]
            tmp = prep.tile([N, TK], FD)

            def bc_tl(src_t):   # [N,T] -> (t,l) view
                return bass.AP(tensor=src_t.tensor, offset=src_t.offset,
                               ap=[src_t.ap[0], [src_t.ap[-1][0], T], [0, K]])

            def bc_lt(src_l):   # [N,K] -> (t,l) view
                return bass.AP(tensor=src_l.tensor, offset=src_l.offset,
                               ap=[src_l.ap[0], [0, T], [src_l.ap[-1][0], K]])

            wx3 = wx[:, :].rearrange("p (t l) -> p t l", l=K)
            wy3 = wy[:, :].rearrange("p (t l) -> p t l", l=K)
            tmp3 = tmp[:, :].rearrange("p (t l) -> p t l", l=K)

            nc.vector.tensor_tensor(out=tmp3, in0=bc_tl(cosT), in1=bc_lt(cent),
                                    op=mybir.AluOpType.mult)
            nc.vector.tensor_tensor(out=wx3, in0=tmp3, in1=bc_tl(x_ap),
                                    op=mybir.AluOpType.add)
            nc.vector.tensor_tensor(out=tmp3, in0=bc_tl(sinT), in1=bc_lt(cent),
                                    op=mybir.AluOpType.mult)
            nc.vector.tensor_tensor(out=wy3, in0=bc_tl(ypos_ap), in1=tmp3,
                                    op=mybir.AluOpType.subtract)

            # ---- stage 2: bounce to DRAM, replicate ----
            nc.sync.dma_start(
                out=bass.AP(tensor=stage_x.tensor, offset=0,
                            ap=[[TK, N], [1, TK]]),
                in_=wx)
            nc.sync.dma_start(
                out=bass.AP(tensor=stage_y.tensor, offset=0,
                            ap=[[TK, N], [1, TK]]),
                in_=wy)
            nc.sync.dma_start(
                out=bass.AP(tensor=stage_r.tensor, offset=0,
                            ap=[[1, N], [1, 1]]),
                in_=rad)

            xj = rep.tile([128, TK], FD)
            yj = rep.tile([128, TK], FD)
            radj = rep.tile([128, 1], FD)
            radi = rep.tile([128, NPAIR], FD)
            for ih in range(2):
                sl = slice(ih * N, (ih + 1) * N)
                nc.sync.dma_start(
                    out=xj[sl, :],
                    in_=bass.AP(tensor=stage_x.tensor, offset=0,
                                ap=[[TK, N], [1, TK]]))
                nc.sync.dma_start(
                    out=yj[sl, :],
                    in_=bass.AP(tensor=stage_y.tensor, offset=0,
                                ap=[[TK, N], [1, TK]]))
                nc.sync.dma_start(
                    out=radj[sl, :],
                    in_=bass.AP(tensor=stage_r.tensor, offset=0,
                                ap=[[1, N], [0, 1]]))
                nc.sync.dma_start(
                    out=radi[sl, :],
                    in_=bass.AP(tensor=stage_r.tensor, offset=HOFF + ih,
                                ap=[[0, N], [2, NPAIR]]))

            wtt = rep.tile([128, NPAIR * T], FD)
            nc.sync.dma_start(out=wtt, in_=wt_in)

            # pd = radi + radj + BUFFER ; invpd = 1/pd
            pd = rep.tile([128, NPAIR], FD)
            nc.vector.tensor_scalar(out=pd, in0=radi, scalar1=radj,
                                    scalar2=float(BUFFER_DIST),
                                    op0=mybir.AluOpType.add,
                                    op1=mybir.AluOpType.add)
            invpd = rep.tile([128, NPAIR], FD)
            nc.vector.reciprocal(out=invpd, in_=pd)

            dminb = acc.tile([128, NPAIR * T], FD)

            def xj_tkl(src):    # [128,(t,k? no t,l)] -> (t,k,l): k bcast
                return bass.AP(tensor=src.tensor, offset=src.offset,
                               ap=[src.ap[0], [K, T], [0, K], [1, K]])

            # ---- stage 3: main loop, two interleaved streams ----
            for g in range(NPAIR // 2):
                ips = (2 * g, 2 * g + 1)
                sfx = ("a", "b")
                xis, yis, dxs, dys, sqxs, sqys, d2s = [], [], [], [], [], [], []
                for s in range(2):
                    ip = ips[s]
                    xi = xiyi.tile([128, TK], FD, tag="xi" + sfx[s])
                    yi = xiyi.tile([128, TK], FD, tag="yi" + sfx[s])
                    off = (HOFF + 2 * ip) * TK
                    for ih in range(2):
                        sl = slice(ih * N, (ih + 1) * N)
                        nc.sync.dma_start(
                            out=xi[sl, :],
                            in_=bass.AP(tensor=stage_x.tensor,
                                        offset=off + ih * TK,
                                        ap=[[0, N], [1, TK]]))
                        nc.sync.dma_start(
                            out=yi[sl, :],
                            in_=bass.AP(tensor=stage_y.tensor,
                                        offset=off + ih * TK,
                                        ap=[[0, N], [1, TK]]))
                    xis.append(xi)
                    yis.append(yi)

                def xi_tkl(src):  # (t,k) -> (t,k,l): l bcast
                    return bass.AP(tensor=src.tensor, offset=src.offset,
                                   ap=[src.ap[0], [K, T], [1, K], [0, K]])

                for s in range(2):
                    dx = work.tile([128, TKL], FD, tag="dx" + sfx[s])
                    dy = work.tile([128, TKL], FD, tag="dy" + sfx[s])
                    dx3 = dx[:, :].rearrange("p (t k l) -> p t k l", k=K, l=K)
                    dy3 = dy[:, :].rearrange("p (t k l) -> p t k l", k=K, l=K)
                    nc.vector.tensor_tensor(out=dx3, in0=xi_tkl(xis[s]),
                                            in1=xj_tkl(xj),
                                            op=mybir.AluOpType.subtract)
                    nc.vector.tensor_tensor(out=dy3, in0=xi_tkl(yis[s]),
                                            in1=xj_tkl(yj),
                                            op=mybir.AluOpType.subtract)
                    dxs.append(dx)
                    dys.append(dy)
                for s in range(2):
                    sqx = work.tile([128, TKL], FD, tag="sqx" + sfx[s])
                    sqy = work.tile([128, TKL], FD, tag="sqy" + sfx[s])
                    nc.scalar.activation(out=sqx, in_=dxs[s],
                                         func=mybir.ActivationFunctionType.Square,
                                         bias=zero128)
                    nc.scalar.activation(out=sqy, in_=dys[s],
                                         func=mybir.ActivationFunctionType.Square,
                                         bias=zero128)
                    sqxs.append(sqx)
                    sqys.append(sqy)
                for s in range(2):
                    d2 = work.tile([128, TKL], FD, tag="d2" + sfx[s])
                    nc.gpsimd.tensor_tensor(out=d2, in0=sqxs[s], in1=sqys[s],
                                            op=mybir.AluOpType.add)
                    d2s.append(d2)
                for s in range(2):
                    ip = ips[s]
                    d23 = d2s[s][:, :].rearrange("p (t kl) -> p t kl",
                                                 kl=K * K)
                    nc.vector.tensor_reduce(
                        out=dminb[:, ip * T:(ip + 1) * T], in_=d23,
                        axis=mybir.AxisListType.X, op=mybir.AluOpType.min)

            # ---- stage 4: finish ----
            nc.vector.tensor_scalar(out=dminb, in0=dminb, scalar1=0.0,
                                    scalar2=0.0, op0=mybir.AluOpType.max,
                                    op1=mybir.AluOpType.add)
            dist = acc.tile([128, NPAIR * T], FD)
            nc.scalar.activation(out=dist, in_=dminb,
                                 func=mybir.ActivationFunctionType.Sqrt,
                                 bias=zero128)
            # q = dist * invpd  (invpd broadcast over t)
            q = acc.tile([128, NPAIR * T], FD)
            q3 = q[:, :].rearrange("p (i t) -> p i t", t=T)
            d3 = dist[:, :].rearrange("p (i t) -> p i t", t=T)
            nc.vector.tensor_tensor(
                out=q3, in0=d3,
                in1=bass.AP(tensor=invpd.tensor, offset=invpd.offset,
                            ap=[invpd.ap[0], [1, NPAIR], [0, T]]),
                op=mybir.AluOpType.mult)
            # pen_neg = min(q - 1, 0)
            nc.vector.tensor_scalar(out=q, in0=q, scalar1=1.0, scalar2=0.0,
                                    op0=mybir.AluOpType.subtract,
                                    op1=mybir.AluOpType.min)
            # weight (mask * ew / BNT) and reduce
            nc.vector.tensor_tensor(out=q, in0=q, in1=wtt,
                                    op=mybir.AluOpType.mult)
            part = acc.tile([128, 1], FD)
            nc.vector.tensor_reduce(out=part, in_=q,
                                    axis=mybir.AxisListType.X,
                                    op=mybir.AluOpType.add)
            nc.sync.dma_start(out=part_out, in_=part)

    nc.compile()
    return nc


def kernel(Y, length, width):
    Y = np.asarray(Y, np.float32)
    length = np.asarray(length, np.float32)
    width = np.asarray(width, np.float32)

    if "nc" not in _CACHE:
        _CACHE["nc"] = _build()
    nc = _CACHE["nc"]

    frac = (2.0 * np.arange(K, dtype=np.float32) / (K - 1) - 1.0).astype(np.float32)
    ew = DECAY_RATE ** np.arange(T, dtype=np.float32)
    ew = ew / ew.sum()

    in_maps = []
    for c in range(NCORES):
        b, h = divmod(c, 2)
        # rotate agents so i-agents are rows [0,32)
        perm = np.r_[h * 32:(h * 32 + N)] % N
        Yb = Y[b][perm].reshape(N, T * D)
        lb = length[b][perm].reshape(N, 1)
        wb = width[b][perm].reshape(N, 1)
        # weight tile: [128 = (parity, j), NPAIR*T]
        wt = np.zeros((128, NPAIR * T), np.float32)
        for p in range(128):
            ih, j = divmod(p, N)
            for ip in range(NPAIR):
                i_loc = 2 * ip + ih          # row in permuted agent axis
                if i_loc != j:               # mask diagonal
                    wt[p, ip * T:(ip + 1) * T] = ew / (B * N * T)
        in_maps.append({
            "y_in": Yb, "len_in": lb, "wid_in": wb,
            "frac_in": frac, "wt_in": wt,
        })

    global _LAST_INMAPS
    _LAST_INMAPS = in_maps
    res = bass_utils.run_bass_kernel_spmd(nc, in_maps,
                                          core_ids=list(range(NCORES)))
    total = 0.0
    for c in range(NCORES):
        total += float(res.results[c]["part_out"].astype(np.float64).sum())
    return np.float32(-total)
